# revision 24
# baseline (speedup 1.0000x reference)
"""GQA + sliding-window attention (B=2, S=2048, E=2048, HQ=16, HKV=4, D=128, W=512).

Sharding: 8 cores = 2 batches x 4 KV-head groups (tensor parallel).
Each core computes its batch's full sequence for one KV head + its 4 Q heads,
plus the (row-sharded) output projection partial; the host sums the 4 partials
per batch (the "all-reduce" done host-side) and adds bo.

v3 (vs v2, 238us):
  - mask application moved off the PE: instead of adding -30000 to the score
    psum via 112 extra mask matmuls, exp runs on the raw scores and the two
    triangular boundary blocks of each e-tile are multiplied by resident 0/1
    masks on the DVE (same zeros in the PV/rowsum inputs, ~6us less PE).
  - e tiles live in one resident [128, ST, 640] sbuf tensor (was a rotating
    pool), so cross-head reuse is plain range tracking.
  - softmax reciprocal uses reciprocal_approx_fast (~5x faster than the
    Newton DVE reciprocal; 18 correct bits is far beyond the bf16 data).
  - PV runs two tiles behind exp in heads 0-2 (one behind in the last head)
    so ScalarE's exp latency never gates the PE.
  - oproj pulled in from kj-3 to kj-2, shrinking the end-of-kernel tail;
    its psum->sbuf stage copies alternate ScalarE/DVE and its dram stores
    alternate the sync/gpsimd queues.
  - input DMA spread over all four engine queues in k-tile order so the
    P1 projection chase never stalls on one queue's descriptor stream.
"""

import os

import numpy as np
import ml_dtypes

import concourse.bass as bass
import concourse.mybir as mybir
import concourse.tile as tile
from concourse.tile import add_dep_helper
from concourse.bass_utils import run_bass_kernel_spmd

B, S, E = 2, 2048, 2048
HQ, HKV, D = 16, 4, 128
WINDOW = 512
ROPE_BASE = 10000.0
N_CORES = 8
GROUP = HQ // HKV          # 4 Q heads per KV head
HD_Q = GROUP * D           # 512
ST = S // 128              # 16 sequence tiles
KTILES = E // 128          # 16 contraction tiles over E
WT = WINDOW // 128         # 4 -> window spans WT+1 = 5 q-tiles

f32 = mybir.dt.float32
bf16 = mybir.dt.bfloat16


def _split_sync_waits(nc, max_waits=1):
    """walrus in this container rejects instructions with more than one
    sync-wait; split extras onto preceding same-engine NoOps."""
    for fn in nc.m.functions:
        for blk in fn.blocks:
            new_insts = []
            for inst in blk.instructions:
                si = getattr(inst, "sync_info", None)
                if si is not None and len(si.on_wait) > max_waits:
                    waits = list(si.on_wait)
                    head, tail = waits[:-max_waits], waits[-max_waits:]
                    for i in range(0, len(head), max_waits):
                        nop = mybir.InstNoOp(
                            name=f"splitwait-{nc.next_id()}",
                            ins=[], outs=[],
                            sync_info=mybir.SyncInfo(
                                on_wait=head[i:i + max_waits], on_update=[]),
                            bass_nofuse=True,
                        )
                        nop.engine = inst.engine
                        new_insts.append(nop)
                    inst.sync_info = mybir.SyncInfo(
                        on_wait=tail, on_update=list(si.on_update))
                new_insts.append(inst)
            blk.instructions[:] = new_insts


def build_kernel(has_bias):
    nc = bass.Bass("TRN2", target_bir_lowering=False, debug=False,
                   num_devices=N_CORES)
    Exp = mybir.ActivationFunctionType.Exp

    xT = nc.dram_tensor("xT", [E, S], bf16, kind="ExternalInput").ap()
    # weights arrive partition-major ([p, t*D+d] = W^T[t*128+p, d]) so each
    # partition's DMA data is contiguous: the natural [E, D] layout yields
    # 256-byte scatter elements and a ~38GB/s transfer that gated P1's
    # first matmul at ~20us.
    wqR = [nc.dram_tensor(f"wq{h}R", [128, KTILES * D], bf16,
                          kind="ExternalInput").ap() for h in range(GROUP)]
    wkR = nc.dram_tensor("wkR", [128, KTILES * D], bf16,
                         kind="ExternalInput").ap()
    wvR = nc.dram_tensor("wvR", [128, KTILES * D], bf16,
                         kind="ExternalInput").ap()
    woT = nc.dram_tensor("woT", [HD_Q, E], bf16, kind="ExternalInput").ap()
    cosT = nc.dram_tensor("cosT", [D, S], bf16, kind="ExternalInput").ap()
    sinT = nc.dram_tensor("sinT", [D, S], bf16, kind="ExternalInput").ap()
    if has_bias:
        bqr = nc.dram_tensor("bqr", [1, HD_Q], bf16, kind="ExternalInput").ap()
        bkr = nc.dram_tensor("bkr", [1, D], bf16, kind="ExternalInput").ap()
        bvr = nc.dram_tensor("bvr", [1, D], bf16, kind="ExternalInput").ap()
    out = nc.dram_tensor("out", [S, E], bf16, kind="ExternalOutput").ap()

    with tile.TileContext(nc) as tc:
        with tc.tile_pool(name="singles", bufs=1) as singles, \
             tc.tile_pool(name="upool", bufs=4) as upool, \
             tc.tile_pool(name="rbpool", bufs=3) as rbpool, \
             tc.tile_pool(name="ostage", bufs=6) as ostage:

            # ---- resident tensors ----
            # weight tiles are flat [128, KTILES*D] so their DMA writes are
            # contiguous 4KB per partition (a column-sliced [128,KT,512]
            # destination scatters into 16 256B pieces and crawls at the
            # descriptor rate; it made P1 stall 11-22us on its stationaries)
            xt = singles.tile([128, KTILES, S], bf16)
            wqh = [singles.tile([128, KTILES * D], bf16, name=f"wqh{h}")
                   for h in range(GROUP)]
            wk = singles.tile([128, KTILES * D], bf16)
            wv = singles.tile([128, KTILES * D], bf16)
            wo = singles.tile([128, GROUP, E], bf16)
            cost = singles.tile([128, S], bf16)
            sint = singles.tile([128, S], bf16)
            qt = singles.tile([128, GROUP, S], bf16)
            kt = singles.tile([128, S], bf16)
            vtsb = singles.tile([128, S], bf16)
            vv = singles.tile([128, ST, D], bf16)
            ot = singles.tile([128, GROUP * ST, D], bf16)
            e_all = singles.tile([128, ST, 640], bf16)
            trimaskP = singles.tile([128, 2, 128], bf16)
            ones128 = singles.tile([128, 128], bf16)
            ident = singles.tile([128, 128], bf16)

            # Input loads. Descriptor ISSUE on one queue is ~650ns per
            # dma_start, so xt's 16 tiles are spread over the three engine
            # queues in k-tile order (P1 chases them k-outer); the weights
            # each phase needs first ride ahead of the later xt tiles on
            # their queue.
            def xtile(eng, t):
                eng.dma_start(out=xt[:, t, :], in_=xT[t * 128:(t + 1) * 128, :])

            # Queues share ~350GB/s aggregate; each queue's transfers run in
            # issue order, so each queue carries its xt tiles back-to-back
            # (k-interleaved across queues => tiles land in k order) and the
            # weights ride the tails, ordered by when each is first needed.
            # The very first transfers are split in half so P1's k=0 inputs
            # beat the ~10us queue spin-up by as much as possible.
            # scalar queue: wk's k0-7 half, xt2, wk's other half (only
            # needed at P1 k=8), xt share, then late weights
            nc.scalar.dma_start(out=wk[:, 0:8 * D], in_=wkR[:, 0:8 * D])
            xtile(nc.scalar, 2)
            nc.scalar.dma_start(out=wk[:, 8 * D:], in_=wkR[:, 8 * D:])
            for t in (5, 8, 11, 14):
                xtile(nc.scalar, t)
            nc.scalar.dma_start(out=wqh[2][:], in_=wqR[2])
            nc.scalar.dma_start(out=wqh[3][:], in_=wqR[3])
            nc.scalar.dma_start(out=wo[:], in_=woT.rearrange("(h p) e -> p h e", p=128))
            # sync queue: xt0 split in half (first moving data), xt share,
            # rope tables (first used at the P1 rope drain, ~42us)
            nc.sync.dma_start(out=xt[:, 0, 0:1024], in_=xT[0:128, 0:1024])
            nc.sync.dma_start(out=xt[:, 0, 1024:2048], in_=xT[0:128, 1024:2048])
            for t in (3, 6, 9, 12, 15):
                xtile(nc.sync, t)
            nc.sync.dma_start(out=sint[:], in_=sinT)
            nc.sync.dma_start(out=cost[:], in_=cosT)
            # gpsimd queue: wq0's k0-7 half, xt1, wq0's other half, xt
            # share, then P2's first weights (after xt13 so no P1 tile
            # queues behind them)
            nc.gpsimd.dma_start(out=wqh[0][:, 0:8 * D], in_=wqR[0][:, 0:8 * D])
            xtile(nc.gpsimd, 1)
            nc.gpsimd.dma_start(out=wqh[0][:, 8 * D:], in_=wqR[0][:, 8 * D:])
            xtile(nc.gpsimd, 4)
            xtile(nc.gpsimd, 7)
            xtile(nc.gpsimd, 10)
            xtile(nc.gpsimd, 13)
            nc.gpsimd.dma_start(out=wqh[1][:], in_=wqR[1])
            nc.gpsimd.dma_start(out=wv[:], in_=wvR)
            bq_t = bk_t = bv_t = onesrow = None
            if has_bias:
                bq_t = singles.tile([1, HD_Q], bf16)
                bk_t = singles.tile([1, D], bf16)
                bv_t = singles.tile([1, D], bf16)
                onesrow = singles.tile([1, 512], bf16)
                nc.sync.dma_start(out=bq_t[:], in_=bqr)
                nc.sync.dma_start(out=bk_t[:], in_=bkr)
                nc.sync.dma_start(out=bv_t[:], in_=bvr)
                nc.gpsimd.memset(onesrow[:], 1.0)

            # 0/1 boundary masks, multiplied into the e tiles after exp
            # (masked score entries then contribute exactly 0 to PV and the
            # rowsum, same as the old exp(-30000) path). Packed as one
            # [128, 2, 128] tile so both boundary blocks of an e tile are
            # masked by a single strided DVE op.
            # slot 0, diag block ST[k(p), q(x)]: keep q >= k -> x - p >= 0.
            # slot 1, off-4 block: keep q - k <= 512 -> p - x >= 0.
            nc.gpsimd.memset(trimaskP[:], 1.0)
            nc.gpsimd.affine_select(
                out=trimaskP[:, 0, :], in_=trimaskP[:, 0, :],
                compare_op=mybir.AluOpType.is_ge,
                fill=0.0, base=0, channel_multiplier=-1, pattern=[[1, 128]])
            nc.gpsimd.affine_select(
                out=trimaskP[:, 1, :], in_=trimaskP[:, 1, :],
                compare_op=mybir.AluOpType.is_ge,
                fill=0.0, base=0, channel_multiplier=1, pattern=[[-1, 128]])
            nc.gpsimd.memset(ones128[:], 1.0)
            nc.gpsimd.memset(ident[:], 1.0)
            nc.gpsimd.affine_select(
                out=ident[:], in_=ident[:], compare_op=mybir.AluOpType.is_equal,
                fill=0.0, base=0, channel_multiplier=-1, pattern=[[1, 128]])

            # PE warmup: the DMA queues take ~10us to deliver the first
            # inputs; burn the tensor engine's p-state ramp on throwaway
            # matmuls meanwhile (operands come from DVE memsets so nothing
            # queues behind gpsimd's DMA issue stream).
            wst = singles.tile([128, 128], bf16)
            wmv = singles.tile([128, 256], bf16)
            nc.vector.memset(wst[:], 0.0)
            nc.vector.memset(wmv[:], 0.0)
            with tc.tile_pool(name="warm", bufs=1, space="PSUM") as warm:
                wps = warm.tile([128, 256], f32)
                for _ in range(10):
                    nc.tensor.matmul(wps[:], wst[:], wmv[:],
                                     start=True, stop=True)

            # ---- projections ----
            def rope_chunk(ps, dst, sl):
                """dst[:, sl] = rope(ps); ps is a psum view whose columns
                correspond to dst's slice sl. ScalarE stages the psum chunk
                to bf16 so the three DVE ops run in cheap 2x sbuf mode; the
                partition-half swap DMAs ride gpsimd's queue. sint here is
                pre-swapped (see _rope_tables): out = tmp*cos + swap64(tmp)*sint."""
                tmp = upool.tile([128, 512], bf16, tag="u")
                tsw = upool.tile([128, 512], bf16, tag="ush")
                u2 = upool.tile([128, 512], bf16, tag="u2")
                nc.scalar.copy(tmp[:], ps)
                nc.gpsimd.dma_start(out=tsw[0:64, :], in_=tmp[64:128, :])
                nc.gpsimd.dma_start(out=tsw[64:128, :], in_=tmp[0:64, :])
                nc.vector.tensor_mul(dst[:, sl], tmp[:], cost[:, sl])
                nc.vector.tensor_mul(u2[:], tsw[:], sint[:, sl])
                nc.vector.tensor_add(dst[:, sl], dst[:, sl], u2[:])

            def bqs(lo, hi):
                return bq_t[:, lo:hi] if has_bias else None

            # P1: K and Q0 k-outer over full-width [128, S] psum tiles, so
            # matmuls chase the xT tile DMAs as they land. Two separate
            # pools so P2 (placed in K's banks) only waits on K's rope
            # drain, overlapping Q0's drain with P2 compute.
            with tc.tile_pool(name="projk", bufs=1, space="PSUM") as projk, \
                 tc.tile_pool(name="projq0", bufs=1, space="PSUM") as projq0:
                p1 = [
                    (lambda k: wk[:, k * D:(k + 1) * D], bk_t, kt[:]),
                    (lambda k: wqh[0][:, k * D:(k + 1) * D], bqs(0, 128),
                     qt[:, 0, :]),
                ]
                tiles = [projk.tile([128, S], f32, tag="pjk", name="pj_k"),
                         projq0.tile([128, S], f32, tag="pjq", name="pj_q0")]
                def p1_mm(ji, k):
                    stf, btile, _ = p1[ji]
                    pt = tiles[ji]
                    for n in range(S // 512):
                        nc.tensor.matmul(
                            pt[:, n * 512:(n + 1) * 512], stf(k),
                            xt[:, k, n * 512:(n + 1) * 512],
                            start=(k == 0),
                            stop=(k == KTILES - 1 and btile is None))

                # interleave K/Q0 per tile, but finish K's last two k's
                # before Q0's: K's rope drain gates P2's psum banks, so
                # completing K ~2us earlier starts P2 ~2us earlier
                for k in range(KTILES - 2):
                    p1_mm(0, k)
                    p1_mm(1, k)
                for k in (KTILES - 2, KTILES - 1):
                    p1_mm(0, k)
                for k in (KTILES - 2, KTILES - 1):
                    p1_mm(1, k)
                if has_bias:
                    for (stf, btile, _), pt in zip(p1, tiles):
                        for n in range(S // 512):
                            nc.tensor.matmul(
                                pt[:, n * 512:(n + 1) * 512], btile,
                                onesrow[0:1, :], start=False, stop=True)
                for (_, _, dst), pt in zip(p1, tiles):
                    for n in range(S // 512):
                        sl = slice(n * 512, (n + 1) * 512)
                        rope_chunk(pt[:, sl], dst, sl)

            # P2: V^T, Q1-Q3 as 512-col chunks through a 4-deep psum pool;
            # chunk drains overlap the next chunk's accumulation. V^T->V
            # transposes run in their own 2-bank pool, spread across the Q1
            # chunk loop so they never stall the projection matmul stream.
            with tc.tile_pool(name="proj2", bufs=4, space="PSUM") as proj2, \
                 tc.tile_pool(name="tpp", bufs=2, space="PSUM") as tpp:

                def transpose_chunk(c):
                    for t in range(4):
                        sm = c * 4 + t
                        tp = tpp.tile([128, 128], bf16, tag="tp")
                        nc.tensor.transpose(
                            tp[:], vtsb[:, sm * 128:(sm + 1) * 128], ident[:])
                        nc.scalar.copy(vv[:, sm, :], tp[:])

                p2 = [
                    (lambda k: wqh[1][:, k * D:(k + 1) * D], bqs(128, 256),
                     qt[:, 1, :]),
                    (lambda k: wv[:, k * D:(k + 1) * D], bv_t, None),
                    (lambda k: wqh[2][:, k * D:(k + 1) * D], bqs(256, 384),
                     qt[:, 2, :]),
                    (lambda k: wqh[3][:, k * D:(k + 1) * D], bqs(384, 512),
                     qt[:, 3, :]),
                ]
                todo_tp = []
                for ji, (stf, btile, dst) in enumerate(p2):
                    for c in range(S // 512):
                        sl = slice(c * 512, (c + 1) * 512)
                        pc = proj2.tile([128, 512], f32, tag="pc")
                        for k in range(KTILES):
                            nc.tensor.matmul(
                                pc[:], stf(k), xt[:, k, sl],
                                start=(k == 0),
                                stop=(k == KTILES - 1 and btile is None))
                        if has_bias:
                            nc.tensor.matmul(
                                pc[:], btile, onesrow[0:1, :],
                                start=False, stop=True)
                        if todo_tp:
                            transpose_chunk(todo_tp.pop(0))
                        if dst is None:
                            nc.scalar.copy(vtsb[:, sl], pc[:])
                            todo_tp.append(c)
                        else:
                            rope_chunk(pc[:], dst, sl)
                for c in todo_tp:
                    transpose_chunk(c)

            # ---- attention + (for last head) output projection ----
            with tc.tile_pool(name="score_psum", bufs=2, space="PSUM") as score_psum, \
                 tc.tile_pool(name="pv_psum", bufs=2, space="PSUM") as pv_psum, \
                 tc.tile_pool(name="out_psum", bufs=2, space="PSUM") as out_psum:

                def oproj(qi):
                    for ch in range(E // 512):
                        pso = out_psum.tile([128, 512], f32, tag="po2")
                        for h in range(GROUP):
                            nc.tensor.matmul(
                                pso[:], ot[:, h * ST + qi, :],
                                wo[:, h, ch * 512:(ch + 1) * 512],
                                start=(h == 0), stop=(h == GROUP - 1))
                        st = ostage.tile([128, 512], bf16, tag="st")
                        # alternate the psum->sbuf stage between ScalarE and
                        # DVE so neither engine's queue gates the out_psum
                        # rotation; stores round-robin all three DMA queues
                        # (a single queue's backlog made the final drain 8us).
                        if ch % 2 == 0:
                            nc.scalar.copy(st[:], pso[:])
                        else:
                            nc.vector.tensor_copy(st[:], pso[:])
                        seng = (nc.sync, nc.gpsimd, nc.scalar)[(qi * 4 + ch) % 3]
                        seng.dma_start(
                            out=out[qi * 128:(qi + 1) * 128,
                                    ch * 512:(ch + 1) * 512],
                            in_=st[:])

                for m in range(GROUP):
                    pair = {}
                    last = (m == GROUP - 1)
                    # PV depth: two tiles behind exp in heads 0-2 (hides
                    # ScalarE's exp latency behind the lighter PE stream),
                    # one behind in the last head to shorten the oproj tail.
                    depth = 1 if last else 2

                    def pv_finish(qi):
                        """PV + rowsum for q-tile qi into half of a paired
                        [128,512] psum bank, laid out [pv_even | pv_odd |
                        rs_even | rs_odd] so that after the odd half ONE
                        contiguous [128,256] reciprocal + multiply normalize
                        both q-tiles at once."""
                        kjs = list(range(max(0, qi - WT), qi + 1))
                        even = (qi % 2 == 0)
                        if even:
                            pair['po'] = pv_psum.tile([128, 512], f32, tag="po",
                                                      name=f"po_{m}_{qi}")
                            pair['pv0'] = None
                        po = pair['po']
                        pvb = 0 if even else 128
                        rsb = 256 if even else 384
                        for j, kjj in enumerate(kjs):
                            off = (qi - kjj) * 128
                            mm = nc.tensor.matmul(
                                po[:, pvb:pvb + 128], vv[:, kjj, :],
                                e_all[:, kjj, off:off + 128],
                                start=(j == 0 and even),
                                stop=(j == len(kjs) - 1),
                                skip_group_check=not even)
                            if j == 0:
                                if even:
                                    pair['pv0'] = mm
                                else:
                                    # odd half relies on the even pv0's
                                    # start=True bank has_written clear
                                    add_dep_helper(mm.ins, pair['pv0'].ins,
                                                   sync=False,
                                                   reason="pair bank clear")
                        for j, kjj in enumerate(kjs):
                            off = (qi - kjj) * 128
                            mm = nc.tensor.matmul(
                                po[:, rsb:rsb + 128], ones128[:],
                                e_all[:, kjj, off:off + 128],
                                start=False, stop=(j == len(kjs) - 1),
                                skip_group_check=True)
                            if j == 0:
                                add_dep_helper(mm.ins, pair['pv0'].ins,
                                               sync=False,
                                               reason="rT after bank clear")
                        if last:
                            # last head normalizes per tile, immediately, so
                            # oproj can trail the pv by just 2 loops (one
                            # less 4.2us oproj slot in the end-of-kernel
                            # tail). 1/rowsum = exp(-ln(x)) on ScalarE.
                            lnq = rbpool.tile([128, 128], f32, tag="lnq")
                            rbq = rbpool.tile([128, 128], f32, tag="rbq")
                            nc.scalar.activation(
                                lnq[:], po[:, rsb:rsb + 128],
                                mybir.ActivationFunctionType.Ln)
                            nc.scalar.activation(
                                rbq[:], lnq[:],
                                mybir.ActivationFunctionType.Exp, scale=-1.0)
                            nc.vector.tensor_mul(
                                ot[:, m * ST + qi, :],
                                po[:, pvb:pvb + 128], rbq[:])
                        elif not even:
                            # pair normalize. ScalarE paces heads 0-2 (exp
                            # 0.67us + ln/exp 0.6us per pair vs the PE's
                            # 0.8us/loop), so alternate pairs put 1/rowsum
                            # on the DVE instead -- split into 4x 64-col
                            # reciprocal pieces (one 1.75us blob would delay
                            # the masks queued behind it and stall PV).
                            rb = rbpool.tile([128, 256], f32, tag="rb")
                            if (qi // 2) % 2 == 0:
                                for j in range(4):
                                    sl4 = slice(j * 64, (j + 1) * 64)
                                    nc.vector.reciprocal(
                                        rb[:, sl4], po[:, 256:512][:, sl4])
                            else:
                                lnb = rbpool.tile([128, 256], f32, tag="lnb")
                                nc.scalar.activation(
                                    lnb[:], po[:, 256:512],
                                    mybir.ActivationFunctionType.Ln)
                                nc.scalar.activation(
                                    rb[:], lnb[:],
                                    mybir.ActivationFunctionType.Exp,
                                    scale=-1.0)
                            nc.vector.tensor_mul(
                                ot[:, m * ST + qi - 1:m * ST + qi + 1, :],
                                po[:, 0:256], rb[:])

                    for kj in range(ST):
                        nw = min(WT + 1, ST - kj)
                        W = 128 * nw
                        q0 = kj * 128
                        pss = score_psum.tile([128, 640], f32, tag="ss")
                        n0 = min(W, 512)
                        nc.tensor.matmul(
                            pss[:, 0:n0], kt[:, q0:q0 + 128],
                            qt[:, m, q0:q0 + n0], start=True, stop=True)
                        if W > 512:
                            nc.tensor.matmul(
                                pss[:, 512:W], kt[:, q0:q0 + 128],
                                qt[:, m, q0 + 512:q0 + W], start=True, stop=True)
                        nc.scalar.activation(e_all[:, kj, 0:W], pss[:, 0:W], Exp)
                        # triangular boundary masks (DVE, off the PE); both
                        # boundary blocks in one strided op when present
                        if W > 512:
                            ev = e_all[:, kj, :].rearrange(
                                "p (a c) -> p a c", c=128)[:, 0:5:4, :]
                            nc.vector.tensor_mul(ev, ev, trimaskP[:])
                        else:
                            nc.vector.tensor_mul(
                                e_all[:, kj, 0:128], e_all[:, kj, 0:128],
                                trimaskP[:, 0, :])
                        if kj >= depth:
                            pv_finish(kj - depth)
                            # with per-tile normalize, ot(kj-2) was ready a
                            # full loop ago -- oproj never waits on it
                            if last and kj >= 2:
                                oproj(kj - 2)
                    for qi in range(ST - depth, ST):
                        pv_finish(qi)
                    if last:
                        oproj(ST - 2)
                        oproj(ST - 1)

    _split_sync_waits(nc)
    return nc


def _rope_tables():
    half = D // 2
    inv_freq = 1.0 / (ROPE_BASE ** (np.arange(half, dtype=np.float64) / half))
    ang = np.arange(S, dtype=np.float64)[:, None] * inv_freq[None, :]  # [S, 64]
    cos = np.cos(ang).T.astype(np.float32)          # [64, S]
    sin = np.sin(ang).T.astype(np.float32)
    cosT = np.concatenate([cos, cos], 0)            # [128, S]
    # sign-folded AND pre-swapped for the swap-first rotate-half formula
    # out = x*cos + swap64(x)*sinT
    sinT = np.concatenate([-sin, sin], 0)
    return np.ascontiguousarray(cosT), np.ascontiguousarray(sinT)


def kernel(x, Wq, bq, Wk, bk, Wv, bv, Wo, bo, **kw):
    x = np.asarray(x, np.float32)
    Wq = np.asarray(Wq, np.float32); bq = np.asarray(bq, np.float32)
    Wk = np.asarray(Wk, np.float32); bk = np.asarray(bk, np.float32)
    Wv = np.asarray(Wv, np.float32); bv = np.asarray(bv, np.float32)
    Wo = np.asarray(Wo, np.float32); bo = np.asarray(bo, np.float32)

    has_bias = bool(np.any(bq) or np.any(bk) or np.any(bv))
    nc = build_kernel(has_bias)

    bff = ml_dtypes.bfloat16
    cosT, sinT = _rope_tables()
    scale = 1.0 / np.sqrt(np.float32(D))

    def pmajor(wT):
        """[E, D] -> partition-major [128, KTILES*D]:
        out[p, t*D+d] = wT[t*128+p, d]."""
        D_ = wT.shape[1]
        return np.ascontiguousarray(
            wT.reshape(KTILES, 128, D_).transpose(1, 0, 2).reshape(
                128, KTILES * D_))

    in_maps = []
    for c in range(N_CORES):
        b, h = c // HKV, c % HKV
        qs = slice(h * HD_Q, (h + 1) * HD_Q)
        ks = slice(h * D, (h + 1) * D)
        wqT_h = Wq[qs].T  # [E, 512]
        m = {
            "xT": np.ascontiguousarray(x[b].T).astype(bff),
            "wkR": pmajor((Wk[ks] * scale).T).astype(bff),
            "wvR": pmajor(Wv[ks].T).astype(bff),
            "woT": np.ascontiguousarray(Wo[:, qs].T).astype(bff),
            "cosT": cosT.astype(bff),
            "sinT": sinT.astype(bff),
        }
        for hh in range(GROUP):
            m[f"wq{hh}R"] = pmajor(
                wqT_h[:, hh * 128:(hh + 1) * 128]).astype(bff)
        if has_bias:
            m["bqr"] = np.ascontiguousarray(bq[qs][None, :]).astype(bff)
            m["bkr"] = np.ascontiguousarray((bk[ks] * scale)[None, :]).astype(bff)
            m["bvr"] = np.ascontiguousarray(bv[ks][None, :]).astype(bff)
        in_maps.append(m)

    res = run_bass_kernel_spmd(nc, in_maps, core_ids=list(range(N_CORES)))
    global LAST_RESULT
    LAST_RESULT = res
    if os.environ.get("BASS_KERNEL_RETIME"):
        # executable is now cached in-process: a second run times
        # transfer + device execution without compile.
        import time
        t0 = time.time()
        run_bass_kernel_spmd(nc, in_maps, core_ids=list(range(N_CORES)))
        print(f"retime run (transfer+exec): {time.time()-t0:.3f}s")

    out_full = np.zeros((B, S, E), np.float32)
    for c in range(N_CORES):
        out_full[c // HKV] += res.results[c]["out"].astype(np.float32)
    out_full += bo[None, None, :]
    return out_full


# revision 27
# speedup vs baseline: 1.0171x; 1.0171x over previous
"""GQA + sliding-window attention (B=2, S=2048, E=2048, HQ=16, HKV=4, D=128, W=512).

Sharding: 8 cores = 2 batches x 4 KV-head groups (tensor parallel).
Each core computes its batch's full sequence for one KV head + its 4 Q heads,
plus the (row-sharded) output projection partial; the host sums the 4 partials
per batch (the "all-reduce" done host-side) and adds bo.

v3 (vs v2, 238us):
  - mask application moved off the PE: instead of adding -30000 to the score
    psum via 112 extra mask matmuls, exp runs on the raw scores and the two
    triangular boundary blocks of each e-tile are multiplied by resident 0/1
    masks on the DVE (same zeros in the PV/rowsum inputs, ~6us less PE).
  - e tiles live in one resident [128, ST, 640] sbuf tensor (was a rotating
    pool), so cross-head reuse is plain range tracking.
  - softmax reciprocal uses reciprocal_approx_fast (~5x faster than the
    Newton DVE reciprocal; 18 correct bits is far beyond the bf16 data).
  - PV runs two tiles behind exp in heads 0-2 (one behind in the last head)
    so ScalarE's exp latency never gates the PE.
  - oproj pulled in from kj-3 to kj-2, shrinking the end-of-kernel tail;
    its psum->sbuf stage copies alternate ScalarE/DVE and its dram stores
    alternate the sync/gpsimd queues.
  - input DMA spread over all four engine queues in k-tile order so the
    P1 projection chase never stalls on one queue's descriptor stream.
"""

import os

import numpy as np
import ml_dtypes

import concourse.bass as bass
import concourse.mybir as mybir
import concourse.tile as tile
from concourse.tile import add_dep_helper
from concourse.bass_utils import run_bass_kernel_spmd

B, S, E = 2, 2048, 2048
HQ, HKV, D = 16, 4, 128
WINDOW = 512
ROPE_BASE = 10000.0
N_CORES = 8
GROUP = HQ // HKV          # 4 Q heads per KV head
HD_Q = GROUP * D           # 512
ST = S // 128              # 16 sequence tiles
KTILES = E // 128          # 16 contraction tiles over E
WT = WINDOW // 128         # 4 -> window spans WT+1 = 5 q-tiles

f32 = mybir.dt.float32
bf16 = mybir.dt.bfloat16


def _split_sync_waits(nc, max_waits=1):
    """walrus in this container rejects instructions with more than one
    sync-wait; split extras onto preceding same-engine NoOps."""
    for fn in nc.m.functions:
        for blk in fn.blocks:
            new_insts = []
            for inst in blk.instructions:
                si = getattr(inst, "sync_info", None)
                if si is not None and len(si.on_wait) > max_waits:
                    waits = list(si.on_wait)
                    head, tail = waits[:-max_waits], waits[-max_waits:]
                    for i in range(0, len(head), max_waits):
                        nop = mybir.InstNoOp(
                            name=f"splitwait-{nc.next_id()}",
                            ins=[], outs=[],
                            sync_info=mybir.SyncInfo(
                                on_wait=head[i:i + max_waits], on_update=[]),
                            bass_nofuse=True,
                        )
                        nop.engine = inst.engine
                        new_insts.append(nop)
                    inst.sync_info = mybir.SyncInfo(
                        on_wait=tail, on_update=list(si.on_update))
                new_insts.append(inst)
            blk.instructions[:] = new_insts


def build_kernel(has_bias):
    nc = bass.Bass("TRN2", target_bir_lowering=False, debug=False,
                   num_devices=N_CORES)
    Exp = mybir.ActivationFunctionType.Exp

    xT = nc.dram_tensor("xT", [E, S], bf16, kind="ExternalInput").ap()
    # weights arrive partition-major ([p, t*D+d] = W^T[t*128+p, d]) so each
    # partition's DMA data is contiguous: the natural [E, D] layout yields
    # 256-byte scatter elements and a ~38GB/s transfer that gated P1's
    # first matmul at ~20us.
    wqR = [nc.dram_tensor(f"wq{h}R", [128, KTILES * D], bf16,
                          kind="ExternalInput").ap() for h in range(GROUP)]
    wkR = nc.dram_tensor("wkR", [128, KTILES * D], bf16,
                         kind="ExternalInput").ap()
    wvR = nc.dram_tensor("wvR", [128, KTILES * D], bf16,
                         kind="ExternalInput").ap()
    woT = nc.dram_tensor("woT", [HD_Q, E], bf16, kind="ExternalInput").ap()
    cosT = nc.dram_tensor("cosT", [D, S], bf16, kind="ExternalInput").ap()
    sinT = nc.dram_tensor("sinT", [D, S], bf16, kind="ExternalInput").ap()
    if has_bias:
        bqr = nc.dram_tensor("bqr", [1, HD_Q], bf16, kind="ExternalInput").ap()
        bkr = nc.dram_tensor("bkr", [1, D], bf16, kind="ExternalInput").ap()
        bvr = nc.dram_tensor("bvr", [1, D], bf16, kind="ExternalInput").ap()
    out = nc.dram_tensor("out", [S, E], bf16, kind="ExternalOutput").ap()

    with tile.TileContext(nc) as tc:
        with tc.tile_pool(name="singles", bufs=1) as singles, \
             tc.tile_pool(name="upool", bufs=4) as upool, \
             tc.tile_pool(name="rbpool", bufs=3) as rbpool, \
             tc.tile_pool(name="ostage", bufs=6) as ostage:

            # ---- resident tensors ----
            # weight tiles are flat [128, KTILES*D] so their DMA writes are
            # contiguous 4KB per partition (a column-sliced [128,KT,512]
            # destination scatters into 16 256B pieces and crawls at the
            # descriptor rate; it made P1 stall 11-22us on its stationaries)
            xt = singles.tile([128, KTILES, S], bf16)
            wqh = [singles.tile([128, KTILES * D], bf16, name=f"wqh{h}")
                   for h in range(GROUP)]
            wk = singles.tile([128, KTILES * D], bf16)
            wv = singles.tile([128, KTILES * D], bf16)
            wo = singles.tile([128, GROUP, E], bf16)
            cost = singles.tile([128, S], bf16)
            sint = singles.tile([128, S], bf16)
            qt = singles.tile([128, GROUP, S], bf16)
            kt = singles.tile([128, S], bf16)
            vtsb = singles.tile([128, S], bf16)
            vv = singles.tile([128, ST, D], bf16)
            ot = singles.tile([128, GROUP * ST, D], bf16)
            e_all = singles.tile([128, ST, 640], bf16)
            trimaskP = singles.tile([128, 2, 128], bf16)
            ones128 = singles.tile([128, 128], bf16)
            ident = singles.tile([128, 128], bf16)

            # Input loads. Descriptor ISSUE on one queue is ~650ns per
            # dma_start, so xt's 16 tiles are spread over the three engine
            # queues in k-tile order (P1 chases them k-outer); the weights
            # each phase needs first ride ahead of the later xt tiles on
            # their queue.
            def xtile(eng, t):
                eng.dma_start(out=xt[:, t, :], in_=xT[t * 128:(t + 1) * 128, :])

            # Queues share ~350GB/s aggregate; each queue's transfers run in
            # issue order, so each queue carries its xt tiles back-to-back
            # (k-interleaved across queues => tiles land in k order) and the
            # weights ride the tails, ordered by when each is first needed.
            # The very first transfers are split in half so P1's k=0 inputs
            # beat the ~10us queue spin-up by as much as possible.
            # scalar queue: wk's k0-7 half, xt2, wk's other half (only
            # needed at P1 k=8), xt share, then late weights
            nc.scalar.dma_start(out=wk[:, 0:8 * D], in_=wkR[:, 0:8 * D])
            xtile(nc.scalar, 2)
            nc.scalar.dma_start(out=wk[:, 8 * D:], in_=wkR[:, 8 * D:])
            for t in (5, 8, 11, 14):
                xtile(nc.scalar, t)
            nc.scalar.dma_start(out=wqh[2][:], in_=wqR[2])
            nc.scalar.dma_start(out=wqh[3][:], in_=wqR[3])
            nc.scalar.dma_start(out=wo[:], in_=woT.rearrange("(h p) e -> p h e", p=128))
            # sync queue: xt0 split in half (first moving data), xt share,
            # rope tables (first used at the P1 rope drain, ~42us)
            nc.sync.dma_start(out=xt[:, 0, 0:1024], in_=xT[0:128, 0:1024])
            nc.sync.dma_start(out=xt[:, 0, 1024:2048], in_=xT[0:128, 1024:2048])
            for t in (3, 6, 9, 12, 15):
                xtile(nc.sync, t)
            nc.sync.dma_start(out=sint[:], in_=sinT)
            nc.sync.dma_start(out=cost[:], in_=cosT)
            # gpsimd queue: wq0's k0-7 half, xt1, wq0's other half, xt
            # share, then P2's first weights (after xt13 so no P1 tile
            # queues behind them)
            nc.gpsimd.dma_start(out=wqh[0][:, 0:8 * D], in_=wqR[0][:, 0:8 * D])
            xtile(nc.gpsimd, 1)
            nc.gpsimd.dma_start(out=wqh[0][:, 8 * D:], in_=wqR[0][:, 8 * D:])
            xtile(nc.gpsimd, 4)
            xtile(nc.gpsimd, 7)
            xtile(nc.gpsimd, 10)
            xtile(nc.gpsimd, 13)
            nc.gpsimd.dma_start(out=wqh[1][:], in_=wqR[1])
            nc.gpsimd.dma_start(out=wv[:], in_=wvR)
            bq_t = bk_t = bv_t = onesrow = None
            if has_bias:
                bq_t = singles.tile([1, HD_Q], bf16)
                bk_t = singles.tile([1, D], bf16)
                bv_t = singles.tile([1, D], bf16)
                onesrow = singles.tile([1, 512], bf16)
                nc.sync.dma_start(out=bq_t[:], in_=bqr)
                nc.sync.dma_start(out=bk_t[:], in_=bkr)
                nc.sync.dma_start(out=bv_t[:], in_=bvr)
                nc.gpsimd.memset(onesrow[:], 1.0)

            # 0/1 boundary masks, multiplied into the e tiles after exp
            # (masked score entries then contribute exactly 0 to PV and the
            # rowsum, same as the old exp(-30000) path). Packed as one
            # [128, 2, 128] tile so both boundary blocks of an e tile are
            # masked by a single strided DVE op.
            # slot 0, diag block ST[k(p), q(x)]: keep q >= k -> x - p >= 0.
            # slot 1, off-4 block: keep q - k <= 512 -> p - x >= 0.
            nc.gpsimd.memset(trimaskP[:], 1.0)
            nc.gpsimd.affine_select(
                out=trimaskP[:, 0, :], in_=trimaskP[:, 0, :],
                compare_op=mybir.AluOpType.is_ge,
                fill=0.0, base=0, channel_multiplier=-1, pattern=[[1, 128]])
            nc.gpsimd.affine_select(
                out=trimaskP[:, 1, :], in_=trimaskP[:, 1, :],
                compare_op=mybir.AluOpType.is_ge,
                fill=0.0, base=0, channel_multiplier=1, pattern=[[-1, 128]])
            nc.gpsimd.memset(ones128[:], 1.0)
            nc.gpsimd.memset(ident[:], 1.0)
            nc.gpsimd.affine_select(
                out=ident[:], in_=ident[:], compare_op=mybir.AluOpType.is_equal,
                fill=0.0, base=0, channel_multiplier=-1, pattern=[[1, 128]])

            # PE warmup: the DMA queues take ~10us to deliver the first
            # inputs; burn the tensor engine's p-state ramp on throwaway
            # matmuls meanwhile (operands come from DVE memsets so nothing
            # queues behind gpsimd's DMA issue stream).
            wst = singles.tile([128, 128], bf16)
            wmv = singles.tile([128, 256], bf16)
            nc.vector.memset(wst[:], 0.0)
            nc.vector.memset(wmv[:], 0.0)
            with tc.tile_pool(name="warm", bufs=1, space="PSUM") as warm:
                wps = warm.tile([128, 256], f32)
                for _ in range(14):
                    nc.tensor.matmul(wps[:], wst[:], wmv[:],
                                     start=True, stop=True)

            # ---- projections ----
            def rope_chunk(ps, dst, sl):
                """dst[:, sl] = rope(ps); ps is a psum view whose columns
                correspond to dst's slice sl. ScalarE stages the psum chunk
                to bf16 so the three DVE ops run in cheap 2x sbuf mode; the
                partition-half swap DMAs ride gpsimd's queue. sint here is
                pre-swapped (see _rope_tables): out = tmp*cos + swap64(tmp)*sint."""
                tmp = upool.tile([128, 512], bf16, tag="u")
                tsw = upool.tile([128, 512], bf16, tag="ush")
                u2 = upool.tile([128, 512], bf16, tag="u2")
                nc.scalar.copy(tmp[:], ps)
                nc.gpsimd.dma_start(out=tsw[0:64, :], in_=tmp[64:128, :])
                nc.gpsimd.dma_start(out=tsw[64:128, :], in_=tmp[0:64, :])
                nc.vector.tensor_mul(dst[:, sl], tmp[:], cost[:, sl])
                nc.vector.tensor_mul(u2[:], tsw[:], sint[:, sl])
                nc.vector.tensor_add(dst[:, sl], dst[:, sl], u2[:])

            def bqs(lo, hi):
                return bq_t[:, lo:hi] if has_bias else None

            # P1: K and Q0 k-outer over full-width [128, S] psum tiles, so
            # matmuls chase the xT tile DMAs as they land. Two separate
            # pools so P2 (placed in K's banks) only waits on K's rope
            # drain, overlapping Q0's drain with P2 compute.
            with tc.tile_pool(name="projk", bufs=1, space="PSUM") as projk, \
                 tc.tile_pool(name="projq0", bufs=1, space="PSUM") as projq0:
                p1 = [
                    (lambda k: wk[:, k * D:(k + 1) * D], bk_t, kt[:]),
                    (lambda k: wqh[0][:, k * D:(k + 1) * D], bqs(0, 128),
                     qt[:, 0, :]),
                ]
                tiles = [projk.tile([128, S], f32, tag="pjk", name="pj_k"),
                         projq0.tile([128, S], f32, tag="pjq", name="pj_q0")]
                def p1_mm(ji, k):
                    stf, btile, _ = p1[ji]
                    pt = tiles[ji]
                    for n in range(S // 512):
                        nc.tensor.matmul(
                            pt[:, n * 512:(n + 1) * 512], stf(k),
                            xt[:, k, n * 512:(n + 1) * 512],
                            start=(k == 0),
                            stop=(k == KTILES - 1 and btile is None))

                # interleave K/Q0 per tile, but finish K's last two k's
                # before Q0's: K's rope drain gates P2's psum banks, so
                # completing K ~2us earlier starts P2 ~2us earlier
                for k in range(KTILES - 2):
                    p1_mm(0, k)
                    p1_mm(1, k)
                for k in (KTILES - 2, KTILES - 1):
                    p1_mm(0, k)
                for k in (KTILES - 2, KTILES - 1):
                    p1_mm(1, k)
                if has_bias:
                    for (stf, btile, _), pt in zip(p1, tiles):
                        for n in range(S // 512):
                            nc.tensor.matmul(
                                pt[:, n * 512:(n + 1) * 512], btile,
                                onesrow[0:1, :], start=False, stop=True)
                for (_, _, dst), pt in zip(p1, tiles):
                    for n in range(S // 512):
                        sl = slice(n * 512, (n + 1) * 512)
                        rope_chunk(pt[:, sl], dst, sl)

            # P2: V^T, Q1-Q3 as 512-col chunks through a 4-deep psum pool;
            # chunk drains overlap the next chunk's accumulation. V^T->V
            # transposes run in their own 2-bank pool, spread across the Q1
            # chunk loop so they never stall the projection matmul stream.
            with tc.tile_pool(name="proj2", bufs=4, space="PSUM") as proj2, \
                 tc.tile_pool(name="tpp", bufs=2, space="PSUM") as tpp:

                def transpose_chunk(c):
                    for t in range(4):
                        sm = c * 4 + t
                        tp = tpp.tile([128, 128], bf16, tag="tp")
                        nc.tensor.transpose(
                            tp[:], vtsb[:, sm * 128:(sm + 1) * 128], ident[:])
                        nc.scalar.copy(vv[:, sm, :], tp[:])

                p2 = [
                    (lambda k: wqh[1][:, k * D:(k + 1) * D], bqs(128, 256),
                     qt[:, 1, :]),
                    (lambda k: wv[:, k * D:(k + 1) * D], bv_t, None),
                    (lambda k: wqh[2][:, k * D:(k + 1) * D], bqs(256, 384),
                     qt[:, 2, :]),
                    (lambda k: wqh[3][:, k * D:(k + 1) * D], bqs(384, 512),
                     qt[:, 3, :]),
                ]
                todo_tp = []
                for ji, (stf, btile, dst) in enumerate(p2):
                    for c in range(S // 512):
                        sl = slice(c * 512, (c + 1) * 512)
                        pc = proj2.tile([128, 512], f32, tag="pc")
                        for k in range(KTILES):
                            nc.tensor.matmul(
                                pc[:], stf(k), xt[:, k, sl],
                                start=(k == 0),
                                stop=(k == KTILES - 1 and btile is None))
                        if has_bias:
                            nc.tensor.matmul(
                                pc[:], btile, onesrow[0:1, :],
                                start=False, stop=True)
                        if todo_tp:
                            transpose_chunk(todo_tp.pop(0))
                        if dst is None:
                            nc.scalar.copy(vtsb[:, sl], pc[:])
                            todo_tp.append(c)
                        else:
                            rope_chunk(pc[:], dst, sl)
                for c in todo_tp:
                    transpose_chunk(c)

            # ---- attention + (for last head) output projection ----
            with tc.tile_pool(name="score_psum", bufs=2, space="PSUM") as score_psum, \
                 tc.tile_pool(name="pv_psum", bufs=2, space="PSUM") as pv_psum, \
                 tc.tile_pool(name="out_psum", bufs=2, space="PSUM") as out_psum:

                def oproj(qi):
                    for ch in range(E // 512):
                        pso = out_psum.tile([128, 512], f32, tag="po2")
                        for h in range(GROUP):
                            nc.tensor.matmul(
                                pso[:], ot[:, h * ST + qi, :],
                                wo[:, h, ch * 512:(ch + 1) * 512],
                                start=(h == 0), stop=(h == GROUP - 1))
                        st = ostage.tile([128, 512], bf16, tag="st")
                        # alternate the psum->sbuf stage between ScalarE and
                        # DVE so neither engine's queue gates the out_psum
                        # rotation; stores round-robin all three DMA queues
                        # (a single queue's backlog made the final drain 8us).
                        if ch % 2 == 0:
                            nc.scalar.copy(st[:], pso[:])
                        else:
                            nc.vector.tensor_copy(st[:], pso[:])
                        seng = (nc.sync, nc.gpsimd, nc.scalar)[(qi * 4 + ch) % 3]
                        seng.dma_start(
                            out=out[qi * 128:(qi + 1) * 128,
                                    ch * 512:(ch + 1) * 512],
                            in_=st[:])

                for m in range(GROUP):
                    pair = {}
                    last = (m == GROUP - 1)
                    # PV depth: two tiles behind exp in heads 0-2 (hides
                    # ScalarE's exp latency behind the lighter PE stream),
                    # one behind in the last head to shorten the oproj tail.
                    depth = 1 if last else 2

                    def pv_finish(qi):
                        """PV + rowsum for q-tile qi into half of a paired
                        [128,512] psum bank, laid out [pv_even | pv_odd |
                        rs_even | rs_odd] so that after the odd half ONE
                        contiguous [128,256] reciprocal + multiply normalize
                        both q-tiles at once."""
                        kjs = list(range(max(0, qi - WT), qi + 1))
                        even = (qi % 2 == 0)
                        if even:
                            pair['po'] = pv_psum.tile([128, 512], f32, tag="po",
                                                      name=f"po_{m}_{qi}")
                            pair['pv0'] = None
                        po = pair['po']
                        pvb = 0 if even else 128
                        rsb = 256 if even else 384
                        for j, kjj in enumerate(kjs):
                            off = (qi - kjj) * 128
                            mm = nc.tensor.matmul(
                                po[:, pvb:pvb + 128], vv[:, kjj, :],
                                e_all[:, kjj, off:off + 128],
                                start=(j == 0 and even),
                                stop=(j == len(kjs) - 1),
                                skip_group_check=not even)
                            if j == 0:
                                if even:
                                    pair['pv0'] = mm
                                else:
                                    # odd half relies on the even pv0's
                                    # start=True bank has_written clear
                                    add_dep_helper(mm.ins, pair['pv0'].ins,
                                                   sync=False,
                                                   reason="pair bank clear")
                        for j, kjj in enumerate(kjs):
                            off = (qi - kjj) * 128
                            mm = nc.tensor.matmul(
                                po[:, rsb:rsb + 128], ones128[:],
                                e_all[:, kjj, off:off + 128],
                                start=False, stop=(j == len(kjs) - 1),
                                skip_group_check=True)
                            if j == 0:
                                add_dep_helper(mm.ins, pair['pv0'].ins,
                                               sync=False,
                                               reason="rT after bank clear")
                        if last:
                            # last head normalizes per tile, immediately, so
                            # oproj can trail the pv by just 2 loops (one
                            # less 4.2us oproj slot in the end-of-kernel
                            # tail). 1/rowsum = exp(-ln(x)) on ScalarE.
                            lnq = rbpool.tile([128, 128], f32, tag="lnq")
                            rbq = rbpool.tile([128, 128], f32, tag="rbq")
                            nc.scalar.activation(
                                lnq[:], po[:, rsb:rsb + 128],
                                mybir.ActivationFunctionType.Ln)
                            nc.scalar.activation(
                                rbq[:], lnq[:],
                                mybir.ActivationFunctionType.Exp, scale=-1.0)
                            nc.vector.tensor_mul(
                                ot[:, m * ST + qi, :],
                                po[:, pvb:pvb + 128], rbq[:])
                        elif not even:
                            # pair normalize; 1/rowsum via ScalarE's
                            # exp(-ln(x)). (Putting it on the DVE -- whole or
                            # split into 64-col pieces -- measured slower:
                            # the op stream delays the masks and stalls PV.)
                            rb = rbpool.tile([128, 256], f32, tag="rb")
                            lnb = rbpool.tile([128, 256], f32, tag="lnb")
                            nc.scalar.activation(
                                lnb[:], po[:, 256:512],
                                mybir.ActivationFunctionType.Ln)
                            nc.scalar.activation(
                                rb[:], lnb[:],
                                mybir.ActivationFunctionType.Exp, scale=-1.0)
                            nc.vector.tensor_mul(
                                ot[:, m * ST + qi - 1:m * ST + qi + 1, :],
                                po[:, 0:256], rb[:])

                    for kj in range(ST):
                        nw = min(WT + 1, ST - kj)
                        W = 128 * nw
                        q0 = kj * 128
                        pss = score_psum.tile([128, 640], f32, tag="ss")
                        n0 = min(W, 512)
                        nc.tensor.matmul(
                            pss[:, 0:n0], kt[:, q0:q0 + 128],
                            qt[:, m, q0:q0 + n0], start=True, stop=True)
                        if W > 512:
                            nc.tensor.matmul(
                                pss[:, 512:W], kt[:, q0:q0 + 128],
                                qt[:, m, q0 + 512:q0 + W], start=True, stop=True)
                        nc.scalar.activation(e_all[:, kj, 0:W], pss[:, 0:W], Exp)
                        # triangular boundary masks (DVE, off the PE).
                        # Two contiguous ops, NOT one step-4 strided AP: a
                        # strided write's subtile deps under-ordered a PV
                        # read once in ~8 runs (absmax flaked 3.6e-3->1.6e-2)
                        nc.vector.tensor_mul(
                            e_all[:, kj, 0:128], e_all[:, kj, 0:128],
                            trimaskP[:, 0, :])
                        if W > 512:
                            nc.vector.tensor_mul(
                                e_all[:, kj, 512:640], e_all[:, kj, 512:640],
                                trimaskP[:, 1, :])
                        if kj >= depth:
                            pv_finish(kj - depth)
                            # with per-tile normalize, ot(kj-2) was ready a
                            # full loop ago -- oproj never waits on it
                            if last and kj >= 2:
                                oproj(kj - 2)
                    for qi in range(ST - depth, ST):
                        pv_finish(qi)
                    if last:
                        oproj(ST - 2)
                        oproj(ST - 1)

    _split_sync_waits(nc)
    return nc


def _rope_tables():
    half = D // 2
    inv_freq = 1.0 / (ROPE_BASE ** (np.arange(half, dtype=np.float64) / half))
    ang = np.arange(S, dtype=np.float64)[:, None] * inv_freq[None, :]  # [S, 64]
    cos = np.cos(ang).T.astype(np.float32)          # [64, S]
    sin = np.sin(ang).T.astype(np.float32)
    cosT = np.concatenate([cos, cos], 0)            # [128, S]
    # sign-folded AND pre-swapped for the swap-first rotate-half formula
    # out = x*cos + swap64(x)*sinT
    sinT = np.concatenate([-sin, sin], 0)
    return np.ascontiguousarray(cosT), np.ascontiguousarray(sinT)


def kernel(x, Wq, bq, Wk, bk, Wv, bv, Wo, bo, **kw):
    x = np.asarray(x, np.float32)
    Wq = np.asarray(Wq, np.float32); bq = np.asarray(bq, np.float32)
    Wk = np.asarray(Wk, np.float32); bk = np.asarray(bk, np.float32)
    Wv = np.asarray(Wv, np.float32); bv = np.asarray(bv, np.float32)
    Wo = np.asarray(Wo, np.float32); bo = np.asarray(bo, np.float32)

    has_bias = bool(np.any(bq) or np.any(bk) or np.any(bv))
    nc = build_kernel(has_bias)

    bff = ml_dtypes.bfloat16
    cosT, sinT = _rope_tables()
    scale = 1.0 / np.sqrt(np.float32(D))

    def pmajor(wT):
        """[E, D] -> partition-major [128, KTILES*D]:
        out[p, t*D+d] = wT[t*128+p, d]."""
        D_ = wT.shape[1]
        return np.ascontiguousarray(
            wT.reshape(KTILES, 128, D_).transpose(1, 0, 2).reshape(
                128, KTILES * D_))

    in_maps = []
    for c in range(N_CORES):
        b, h = c // HKV, c % HKV
        qs = slice(h * HD_Q, (h + 1) * HD_Q)
        ks = slice(h * D, (h + 1) * D)
        wqT_h = Wq[qs].T  # [E, 512]
        m = {
            "xT": np.ascontiguousarray(x[b].T).astype(bff),
            "wkR": pmajor((Wk[ks] * scale).T).astype(bff),
            "wvR": pmajor(Wv[ks].T).astype(bff),
            "woT": np.ascontiguousarray(Wo[:, qs].T).astype(bff),
            "cosT": cosT.astype(bff),
            "sinT": sinT.astype(bff),
        }
        for hh in range(GROUP):
            m[f"wq{hh}R"] = pmajor(
                wqT_h[:, hh * 128:(hh + 1) * 128]).astype(bff)
        if has_bias:
            m["bqr"] = np.ascontiguousarray(bq[qs][None, :]).astype(bff)
            m["bkr"] = np.ascontiguousarray((bk[ks] * scale)[None, :]).astype(bff)
            m["bvr"] = np.ascontiguousarray(bv[ks][None, :]).astype(bff)
        in_maps.append(m)

    res = run_bass_kernel_spmd(nc, in_maps, core_ids=list(range(N_CORES)))
    global LAST_RESULT
    LAST_RESULT = res
    if os.environ.get("BASS_KERNEL_RETIME"):
        # executable is now cached in-process: a second run times
        # transfer + device execution without compile.
        import time
        t0 = time.time()
        run_bass_kernel_spmd(nc, in_maps, core_ids=list(range(N_CORES)))
        print(f"retime run (transfer+exec): {time.time()-t0:.3f}s")

    out_full = np.zeros((B, S, E), np.float32)
    for c in range(N_CORES):
        out_full[c // HKV] += res.results[c]["out"].astype(np.float32)
    out_full += bo[None, None, :]
    return out_full
